# revision 1
# baseline (speedup 1.0000x reference)
"""TRN2 Bass kernel for batched dot-product attention (no scale, eval mode).

reference:
    score   = einsum('bqd,bvd->bqv', query, value)      # B=16, L=2048, D=1024
    attn    = softmax(score, axis=-1)
    context = einsum('bqv,bvd->bqd', attn, value)

Sharding: data-parallel over batch; each of 8 NeuronCores handles 2 batch
elements, no communication. Inputs are pre-cast to fp16 on the host; matmuls
run fp16 with fp32 PSUM accumulation.

Key structure (final):
  - Q and P transposes via the DMA XBAR (dma_start_transpose), BOTH issued
    from the SP (sync) engine only: the XBAR completion semaphores (DMAHW
    ring) are shared round-robin across issuing engines, so transposes from
    two engines race the ring and consumers can observe stale tiles. One
    issuing engine keeps the ring increments in program order (hard-won HW
    lesson). SP is otherwise idle, so the ~1.3us engine-blocking XBAR calls
    never delay the exp chain.
  - V transposes on the PE (identity matmul) + DVE copy out of PSUM: V is
    the bulk transpose volume and the XBAR descriptor path is far too slow /
    latency-chained at startup for 2MB per batch.
  - PE instructions are explicitly chained in emission order (sync=False dep
    edges): the tile scheduler otherwise reorders PE instructions and can
    split PSUM accumulation groups, which corrupts results on HW (CoreSim's
    per-address PSUM model tolerates it, hardware does not).
  - MM1 writes four separate 1-bank PSUM chunk tiles (not one 4-bank tile)
    so the WAR edges (rowmax/exp of tile s vs MM1 of tile s+1) resolve per
    chunk and never stall the PE.
  - Output stores run on gpsimd. The two normalization multiplies are
    split across engines (dch0 on DVE, dch1 on ACT) and EMITTED ONE STEP
    AFTER their MM2 (at the head of the next step's engine programs): when
    they queue behind the same step's rowmax/exp chains, the single-buffer
    psO WAR release throttles the next tile's MM2 into a ~1us/step stall.
  - rowsum comes from a DVE reduce over the fp16 P tile (no ACT accum_out):
    fewer ACT instructions and the psS chunks' reader set stays {max, exp}.

Per-core per-batch plan:
  - Vn natural fp16 (gpsimd DMA), VT via PE transposes (interleaved with the
    first tile's MM1 chunks for batch 0; dripped 2 v-tiles/step for the next
    batch)
  - per 128-row q-tile (1-deep software pipeline; MM2 lags one tile):
      QT via XBAR from DRAM (ACT queue, issued one step ahead)
      MM1: S = QT.T @ VT -> 4x512 PSUM chunks, per-chunk rowmax on DVE
      softmax: exp(S - max) on ACT (fp16 P, fused rowsum), then PT via XBAR
      MM2: O = PT.T @ Vn -> PSUM, scale by 1/rowsum (gpsimd, fp16), store
"""

from contextlib import ExitStack

import numpy as np

import concourse.tile as tile
from concourse import bacc, mybir
from concourse.bass import _add_dep_helper
from concourse.masks import make_identity
from concourse.bass_utils import run_bass_kernel_spmd

B, LQ, LV, D = 16, 2048, 2048, 1024
NCORES = 8
BPC = B // NCORES  # batches per core
P = 128
NQT = LQ // P  # 16 q tiles
NVT = LV // P  # 16 v tiles
ND = D // P  # 8 d tiles
VCH = 512  # MM1 matmul group width (one bank of f32)
SCH = 1024  # softmax chunk width (one 2-bank psS tile)
NCH = LV // SCH  # 2
DCH = 512  # MM2 PSUM chunk
NDCH = D // DCH  # 2

f32 = mybir.dt.float32
f16 = mybir.dt.float16
EXP = mybir.ActivationFunctionType.Exp
AX = mybir.AxisListType.X


def build_nc():
    nc = bacc.Bacc("TRN2", target_bir_lowering=False, debug=False)
    q_d = nc.dram_tensor("q16", [BPC, LQ, D], f16, kind="ExternalInput").ap()
    v_d = nc.dram_tensor("v16", [BPC, LV, D], f16, kind="ExternalInput").ap()
    o_d = nc.dram_tensor("o", [BPC, LQ, D], f16, kind="ExternalOutput").ap()

    prev_pe = [None]

    def chain(inst):
        """Order every PE instruction after the previous one (scheduler-order
        edge only; same-engine, so no runtime semaphore is needed)."""
        if prev_pe[0] is not None:
            _add_dep_helper(inst.ins, prev_pe[0].ins, sync=False, reason="pe-order")
        prev_pe[0] = inst
        return inst

    with tile.TileContext(nc) as tc, ExitStack() as ctx:
        const = ctx.enter_context(tc.tile_pool(name="const", bufs=1))
        vtp = ctx.enter_context(tc.tile_pool(name="vtp", bufs=2))
        vnp = ctx.enter_context(tc.tile_pool(name="vnp", bufs=2))
        qtp = ctx.enter_context(tc.tile_pool(name="qtp", bufs=3))
        pp = ctx.enter_context(tc.tile_pool(name="pp", bufs=2))
        ptp = ctx.enter_context(tc.tile_pool(name="ptp", bufs=2))
        outp = ctx.enter_context(tc.tile_pool(name="outp", bufs=2))
        statp = ctx.enter_context(tc.tile_pool(name="statp", bufs=3))
        psumS = ctx.enter_context(tc.tile_pool(name="psumS", bufs=1, space="PSUM"))
        psumO = ctx.enter_context(tc.tile_pool(name="psumO", bufs=1, space="PSUM"))
        psumT = ctx.enter_context(tc.tile_pool(name="psumT", bufs=2, space="PSUM"))

        ident = const.tile([P, P], f16)
        make_identity(nc, ident)

        def alloc_v(b):
            Vn = vnp.tile([P, NVT, D], f16, tag="Vn", name=f"Vn{b}")
            VT = vtp.tile([P, ND, LV], f16, tag="VT", name=f"VT{b}")
            return Vn, VT

        def load_vn(b, Vn, j):
            nc.gpsimd.dma_start(out=Vn[:, j, :], in_=v_d[b, j * P : (j + 1) * P, :])

        def trans_v(b, Vn, VT, j):
            """VT[d, k, j*P+v'] = V[j*P+v', k*P+d] via PE transpose + DVE copy."""
            pst = psumT.tile([P, ND, P], f16, tag="pst", name=f"pst{b}_{j}")
            for k in range(ND):
                chain(
                    nc.tensor.transpose(
                        pst[:, k, :], Vn[:, j, k * P : (k + 1) * P], ident
                    )
                )
            nc.vector.tensor_copy(VT[:, :, j * P : (j + 1) * P], pst)

        def issue_qt(b, qi):
            """QT[d', k, q] = Q[qi*P+q, k*P+d'] via XBAR from DRAM (sync)."""
            QT = qtp.tile([P, ND, P], f16, tag="QT", name=f"QT{b}_{qi}")
            nc.sync.dma_start_transpose(QT, q_d[b, qi * P : (qi + 1) * P, :])
            return QT

        def head(b, qi, VT, QT, prologue=None):
            """S = Q @ V^T into two 2-bank PSUM chunks; per-chunk row maxes.
            Matmul accumulation groups stay one bank (512 f32) wide."""
            psS = []
            stats = statp.tile([P, NCH], f32, tag="stats", name=f"st{b}_{qi}")
            for n in range(NCH):
                ps = psumS.tile([P, SCH], f32, tag=f"psS{n}", name=f"psS{b}_{qi}_{n}")
                psS.append(ps)
                for h2 in range(2):
                    if prologue is not None:
                        prologue(2 * n + h2)
                    v0 = n * SCH + h2 * VCH
                    for k in range(ND):
                        chain(
                            nc.tensor.matmul(
                                ps[:, h2 * VCH : (h2 + 1) * VCH],
                                QT[:, k, :],
                                VT[:, k, v0 : v0 + VCH],
                                start=(k == 0),
                                stop=(k == ND - 1),
                            )
                        )
                nc.vector.reduce_max(stats[:, n : n + 1], ps, axis=AX)
            return psS, stats

        def softmax(b, qi, psS, stats):
            """exp(S - max) -> fp16 P (fused rowsum); PT via XBAR; 1/rowsum."""
            negmax = statp.tile([P, 1], f32, tag="negmax", name=f"nm{b}_{qi}")
            nc.vector.reduce_max(negmax, stats, axis=AX, negate=True)
            Pt = pp.tile([P, LV], f16, tag="P", name=f"P{b}_{qi}")
            PT = ptp.tile([P, NVT, P], f16, tag="PT", name=f"PT{b}_{qi}")
            for n in range(NCH):
                nc.scalar.activation(
                    Pt[:, n * SCH : (n + 1) * SCH],
                    psS[n],
                    EXP,
                    bias=negmax,
                )
            for h in range(2):
                # PT[v', j, q] = P[q, j*P+v'], one 1024-wide half per call,
                # on sync (the only XBAR engine), gated on exp chunk h.
                nc.sync.dma_start_transpose(
                    PT[:, h * ND : (h + 1) * ND, :],
                    Pt[:, h * SCH : (h + 1) * SCH],
                )
            rowsum = statp.tile([P, 1], f32, tag="rowsum", name=f"rs{b}_{qi}")
            nc.vector.reduce_sum(rowsum, Pt, axis=AX)
            rinv = statp.tile([P, 1], f32, tag="rinv", name=f"ri{b}_{qi}")
            nc.vector.reciprocal(rinv, rowsum)
            return PT, rinv

        def tail_mm(b, qi, PT, rinv, Vn):
            """O = P @ V into PSUM; normalize/store deferred one step."""
            psO = psumO.tile([P, D], f32, tag="psO", name=f"psO{b}_{qi}")
            for dch in range(NDCH):
                sl = slice(dch * DCH, (dch + 1) * DCH)
                for j in range(NVT):
                    chain(
                        nc.tensor.matmul(
                            psO[:, sl],
                            PT[:, j, :],
                            Vn[:, j, sl],
                            start=(j == 0),
                            stop=(j == NVT - 1),
                        )
                    )
            return psO

        def tail_fin(b, qi, psO, rinv):
            """Normalize to fp16 (dch0 on DVE, dch1 on ACT), store (gpsimd).
            Emitted at the head of the NEXT step's engine programs so the
            muls run the moment MM2 finishes, releasing psO early."""
            out16 = outp.tile([P, D], f16, tag="out", name=f"o{b}_{qi}")
            nc.vector.tensor_scalar_mul(out16[:, :DCH], psO[:, :DCH], rinv)
            nc.scalar.mul(out16[:, DCH:], psO[:, DCH:], rinv)
            nc.gpsimd.dma_start(o_d[b, qi * P : (qi + 1) * P, :], out16)

        cur_v = alloc_v(0)
        QT_next = issue_qt(0, 0)
        for j in range(NVT):
            load_vn(0, cur_v[0], j)
        nxt_v = None
        pending = None
        fin = None
        NS = BPC * NQT

        def b0_prologue(n):
            # interleave batch-0 V transposes with the first tile's chunks
            for j in range(4 * n, 4 * n + 4):
                trans_v(0, *cur_v, j)

        for s in range(NS):
            b, qi = divmod(s, NQT)
            Vn, VT = cur_v
            QT = QT_next
            if fin is not None:
                tail_fin(*fin)
                fin = None
            if s + 1 < NS:
                QT_next = issue_qt(*divmod(s + 1, NQT))
            if b + 1 < BPC:
                if qi == 2:
                    nxt_v = alloc_v(b + 1)
                if 2 <= qi < 10:
                    for j in ((qi - 2) * 2, (qi - 2) * 2 + 1):
                        load_vn(b + 1, nxt_v[0], j)
                if 6 <= qi < 14:
                    for j in ((qi - 6) * 2, (qi - 6) * 2 + 1):
                        trans_v(b + 1, *nxt_v, j)
            psS, stats = head(b, qi, VT, QT, prologue=b0_prologue if s == 0 else None)
            PT, rinv = softmax(b, qi, psS, stats)
            if pending is not None:
                pb, pq, pPT, prinv, pVn = pending
                psO = tail_mm(pb, pq, pPT, prinv, pVn)
                fin = (pb, pq, psO, prinv)
            pending = (b, qi, PT, rinv, Vn)
            if qi == NQT - 1 and b + 1 < BPC:
                cur_v = nxt_v
        if fin is not None:
            tail_fin(*fin)
        pb, pq, pPT, prinv, pVn = pending
        psO = tail_mm(pb, pq, pPT, prinv, pVn)
        tail_fin(pb, pq, psO, prinv)

    nc.compile()
    return nc


_NC_CACHE = None


def _get_nc():
    global _NC_CACHE
    if _NC_CACHE is None:
        _NC_CACHE = build_nc()
    return _NC_CACHE


def kernel(query: np.ndarray, value: np.ndarray) -> np.ndarray:
    query = np.asarray(query)
    value = np.asarray(value)
    assert query.shape == (B, LQ, D) and value.shape == (B, LV, D)
    q16 = np.ascontiguousarray(query.astype(np.float16))
    v16 = np.ascontiguousarray(value.astype(np.float16))
    nc = _get_nc()
    in_maps = [
        {
            "q16": q16[i * BPC : (i + 1) * BPC],
            "v16": v16[i * BPC : (i + 1) * BPC],
        }
        for i in range(NCORES)
    ]
    res = run_bass_kernel_spmd(nc, in_maps, list(range(NCORES)))
    out = np.concatenate(
        [res.results[i]["o"].astype(np.float32) for i in range(NCORES)], axis=0
    )
    return out



# revision 3
# speedup vs baseline: 1.0342x; 1.0342x over previous
"""TRN2 Bass kernel for batched dot-product attention (no scale, eval mode).

reference:
    score   = einsum('bqd,bvd->bqv', query, value)      # B=16, L=2048, D=1024
    attn    = softmax(score, axis=-1)
    context = einsum('bqv,bvd->bqd', attn, value)

Sharding: data-parallel over batch; each of 8 NeuronCores handles 2 batch
elements, no communication. Inputs are pre-cast to fp16 on the host; matmuls
run fp16 with fp32 PSUM accumulation.

Key structure (v2 — startup/tail/transpose-offload rework of the v1 design):
  - All XBAR (dma_start_transpose) calls issue from the SP (sync) engine
    only: the XBAR completion semaphores (DMAHW ring) are shared round-robin
    across issuing engines, so transposes from two engines race the ring and
    consumers can observe stale tiles (hard-won HW lesson from v1).
  - Tile serializes XBAR transposes against regular DMAs in epochs: each
    XBAR waits for every previously-scheduled regular DMA and vice versa.
    The per-step DMA chain is therefore budgeted: [store+loads] -> QT xbar
    -> (VT xbar) -> PT xbar x2, which fits inside the 13.7us PE step.
  - Batch-0 V transposes run on the PE (identity matmul + DVE copy out of
    PSUM), interleaved with the first tile's MM1 chunks: V arrives at HBM
    rate (~11us) and the PE does useful transpose work while waiting, which
    also warms the HAM clock gate.
  - Batch-1 V transposes instead go through the XBAR straight from DRAM
    (no Vn dependency), dripped 2 v-tiles per step during batch 0: this
    removes 128 PE transposes (~14us of PE time) from the steady state.
  - Batch-0 Vn loads issue from SP (HWDGE) — SWDGE (gpsimd) adds ~5.5us of
    ring latency which sat on the v1 critical path at startup. Batch-1 Vn
    loads stay on gpsimd (huge slack), 2 v-tiles per call, scheduled in the
    same DMA epoch as the output store of the step.
  - Q-tile-0's transpose runs on the PE from a natural Qn load, so the
    first MM1 chunk is not gated on any XBAR call at startup.
  - PE instructions are explicitly chained in emission order (sync=False
    dep edges): the tile scheduler otherwise reorders PE instructions and
    can split PSUM accumulation groups, which corrupts results on HW
    (CoreSim's per-address PSUM model tolerates it, hardware does not).
  - MM1 writes four separate 1-bank PSUM chunk tiles (not one 4-bank tile)
    so the WAR edges (rowmax/exp of tile s vs MM1 of tile s+1) resolve per
    chunk and never stall the PE.
  - MM2 writes two separate 1-bank psO chunk tiles; the two normalization
    multiplies are split across engines (dch0 on DVE, dch1 on ACT) and
    EMITTED ONE STEP AFTER their MM2 (at the head of the next step's engine
    programs): when they queue behind the same step's rowmax/exp chains,
    the psO WAR release throttles the next tile's MM2 into a ~1us/step
    stall. Per-chunk psO lets the final tile's dch0 normalization overlap
    its dch1 matmuls instead of waiting for all of them.
  - Output stores run on ACT (HWDGE): the v1 gpsimd (SWDGE) stores cost a
    ~6.4us ring-drain after the last tile.
  - rowsum comes from a DVE reduce over the fp16 P tile (no ACT accum_out):
    fewer ACT instructions and the psS chunks' reader set stays {max, exp}.

Per-core per-batch plan:
  - batch 0: Qn0 + Vn natural fp16 via SP HWDGE; QT0 and VT via PE
    transposes (VT interleaved with the first tile's MM1 chunks)
  - batch 1: VT via XBAR from DRAM (2 v-tiles/step, qi 1..8 of batch 0),
    Vn natural via gpsimd (2 v-tiles/call, qi 8..15 of batch 0)
  - per 128-row q-tile (1-deep software pipeline; MM2 lags one tile):
      QT via XBAR from DRAM (sync queue, issued one step ahead)
      MM1: S = QT.T @ VT -> 4x512 PSUM chunks, per-chunk rowmax on DVE
      softmax: exp(S - max) on ACT (fp16 P, fused rowsum), then PT via XBAR
      MM2: O = PT.T @ Vn -> 2x512 PSUM chunks, scale by 1/rowsum, store
"""

from contextlib import ExitStack

import numpy as np

import concourse.tile as tile
from concourse import bacc, mybir
from concourse.bass import _add_dep_helper
from concourse.masks import make_identity
from concourse.bass_utils import run_bass_kernel_spmd

B, LQ, LV, D = 16, 2048, 2048, 1024
NCORES = 8
BPC = B // NCORES  # batches per core
P = 128
NQT = LQ // P  # 16 q tiles
NVT = LV // P  # 16 v tiles
ND = D // P  # 8 d tiles
VCH = 512  # MM1 matmul group width (one bank of f32)
SCH = 1024  # softmax chunk width (one 2-bank psS tile)
NCH = LV // SCH  # 2
DCH = 512  # MM2 PSUM chunk
NDCH = D // DCH  # 2

f32 = mybir.dt.float32
f16 = mybir.dt.float16
EXP = mybir.ActivationFunctionType.Exp
AX = mybir.AxisListType.X


def build_nc():
    nc = bacc.Bacc("TRN2", target_bir_lowering=False, debug=False)
    q_d = nc.dram_tensor("q16", [BPC, LQ, D], f16, kind="ExternalInput").ap()
    v_d = nc.dram_tensor("v16", [BPC, LV, D], f16, kind="ExternalInput").ap()
    o_d = nc.dram_tensor("o", [BPC, LQ, D], f16, kind="ExternalOutput").ap()

    prev_pe = [None]

    def chain(inst):
        """Order every PE instruction after the previous one (scheduler-order
        edge only; same-engine, so no runtime semaphore is needed)."""
        if prev_pe[0] is not None:
            _add_dep_helper(inst.ins, prev_pe[0].ins, sync=False, reason="pe-order")
        prev_pe[0] = inst
        return inst

    with tile.TileContext(nc) as tc, ExitStack() as ctx:
        const = ctx.enter_context(tc.tile_pool(name="const", bufs=1))
        vtp = ctx.enter_context(tc.tile_pool(name="vtp", bufs=2))
        vnp = ctx.enter_context(tc.tile_pool(name="vnp", bufs=2))
        qtp = ctx.enter_context(tc.tile_pool(name="qtp", bufs=4))
        qnp = ctx.enter_context(tc.tile_pool(name="qnp", bufs=1))
        pp = ctx.enter_context(tc.tile_pool(name="pp", bufs=2))
        ptp = ctx.enter_context(tc.tile_pool(name="ptp", bufs=2))
        outp = ctx.enter_context(tc.tile_pool(name="outp", bufs=2))
        statp = ctx.enter_context(tc.tile_pool(name="statp", bufs=3))
        psumS = ctx.enter_context(tc.tile_pool(name="psumS", bufs=1, space="PSUM"))
        psumO = ctx.enter_context(tc.tile_pool(name="psumO", bufs=1, space="PSUM"))
        psumT = ctx.enter_context(tc.tile_pool(name="psumT", bufs=2, space="PSUM"))

        ident = const.tile([P, P], f16)
        make_identity(nc, ident)

        def alloc_v(b):
            Vn = vnp.tile([P, NVT, D], f16, tag="Vn", name=f"Vn{b}")
            VT = vtp.tile([P, ND, LV], f16, tag="VT", name=f"VT{b}")
            return Vn, VT

        def trans_v(b, Vn, VT, j):
            """VT[d, k, j*P+v'] = V[j*P+v', k*P+d] via PE transpose + DVE copy."""
            pst = psumT.tile([P, ND, P], f16, tag="pst", name=f"pst{b}_{j}")
            for k in range(ND):
                chain(
                    nc.tensor.transpose(
                        pst[:, k, :], Vn[:, j, k * P : (k + 1) * P], ident
                    )
                )
            nc.vector.tensor_copy(VT[:, :, j * P : (j + 1) * P], pst)

        def issue_qt(b, qi):
            """QT[d', k, q] = Q[qi*P+q, k*P+d'] via XBAR from DRAM (sync)."""
            QT = qtp.tile([P, ND, P], f16, tag="QT", name=f"QT{b}_{qi}")
            nc.sync.dma_start_transpose(QT, q_d[b, qi * P : (qi + 1) * P, :])
            return QT

        def issue_vt_xbar(b, VT, j):
            """VT[:, :, j*P:(j+2)*P] for two v-tiles via XBAR from DRAM."""
            nc.sync.dma_start_transpose(
                VT[:, :, j * P : (j + 2) * P],
                v_d[b, j * P : (j + 2) * P, :],
            )

        def load_vn_pair(b, Vn, j):
            """Natural-layout V, two v-tiles per SWDGE call (gpsimd)."""
            nc.gpsimd.dma_start(
                out=Vn[:, j : j + 2, :],
                in_=v_d[b, j * P : (j + 2) * P, :].rearrange(
                    "(t p) d -> p t d", p=P
                ),
            )

        def head(b, qi, VT, QT, prologue=None):
            """S = Q @ V^T into two 2-bank PSUM chunks; per-chunk row maxes.
            Matmul accumulation groups stay one bank (512 f32) wide."""
            psS = []
            stats = statp.tile([P, NCH], f32, tag="stats", name=f"st{b}_{qi}")
            for n in range(NCH):
                ps = psumS.tile([P, SCH], f32, tag=f"psS{n}", name=f"psS{b}_{qi}_{n}")
                psS.append(ps)
                for h2 in range(2):
                    if prologue is not None:
                        prologue(2 * n + h2)
                    v0 = n * SCH + h2 * VCH
                    for k in range(ND):
                        chain(
                            nc.tensor.matmul(
                                ps[:, h2 * VCH : (h2 + 1) * VCH],
                                QT[:, k, :],
                                VT[:, k, v0 : v0 + VCH],
                                start=(k == 0),
                                stop=(k == ND - 1),
                            )
                        )
                nc.vector.reduce_max(stats[:, n : n + 1], ps, axis=AX)
            return psS, stats

        def softmax(b, qi, psS, stats):
            """exp(S - max) -> fp16 P (fused rowsum); PT via XBAR; 1/rowsum."""
            negmax = statp.tile([P, 1], f32, tag="negmax", name=f"nm{b}_{qi}")
            nc.vector.reduce_max(negmax, stats, axis=AX, negate=True)
            Pt = pp.tile([P, LV], f16, tag="P", name=f"P{b}_{qi}")
            PT = ptp.tile([P, NVT, P], f16, tag="PT", name=f"PT{b}_{qi}")
            for n in range(NCH):
                nc.scalar.activation(
                    Pt[:, n * SCH : (n + 1) * SCH],
                    psS[n],
                    EXP,
                    bias=negmax,
                )
            for h in range(2):
                # PT[v', j, q] = P[q, j*P+v'], one 1024-wide half per call,
                # on sync (the only XBAR engine), gated on exp chunk h.
                nc.sync.dma_start_transpose(
                    PT[:, h * ND : (h + 1) * ND, :],
                    Pt[:, h * SCH : (h + 1) * SCH],
                )
            rowsum = statp.tile([P, 1], f32, tag="rowsum", name=f"rs{b}_{qi}")
            nc.vector.reduce_sum(rowsum, Pt, axis=AX)
            rinv = statp.tile([P, 1], f32, tag="rinv", name=f"ri{b}_{qi}")
            nc.vector.reciprocal(rinv, rowsum)
            return PT, rinv

        def tail_mm(b, qi, PT, rinv, Vn):
            """O = P @ V into two 1-bank PSUM chunks; normalize/store deferred
            one step. Per-chunk tiles let WAR edges resolve per chunk."""
            psO = []
            for dch in range(NDCH):
                ps = psumO.tile(
                    [P, DCH], f32, tag=f"psO{dch}", name=f"psO{b}_{qi}_{dch}"
                )
                psO.append(ps)
                sl = slice(dch * DCH, (dch + 1) * DCH)
                for j in range(NVT):
                    chain(
                        nc.tensor.matmul(
                            ps,
                            PT[:, j, :],
                            Vn[:, j, sl],
                            start=(j == 0),
                            stop=(j == NVT - 1),
                        )
                    )
            return psO

        def tail_fin(b, qi, psO, rinv):
            """Normalize to fp16 (dch0 on DVE, dch1 on ACT), store (ACT HWDGE).
            Emitted at the head of the NEXT step's engine programs so the
            muls run the moment MM2 finishes, releasing psO early."""
            out16 = outp.tile([P, D], f16, tag="out", name=f"o{b}_{qi}")
            nc.vector.tensor_scalar_mul(out16[:, :DCH], psO[0], rinv)
            nc.scalar.mul(out16[:, DCH:], psO[1], rinv)
            nc.scalar.dma_start(out=o_d[b, qi * P : (qi + 1) * P, :], in_=out16)

        # --- batch-0 startup -------------------------------------------------
        cur_v = alloc_v(0)
        # Qn tile 0 natural + batch-0 Vn, all on SP HWDGE (low latency).
        Qn0 = qnp.tile([P, D], f16, tag="Qn0", name="Qn0")
        nc.sync.dma_start(out=Qn0, in_=q_d[0, 0:P, :])
        for j in range(NVT):
            nc.sync.dma_start(
                out=cur_v[0][:, j, :], in_=v_d[0, j * P : (j + 1) * P, :]
            )
        # QT for (0, 0) via PE transpose from Qn0 — no XBAR on the critical
        # path at startup.
        QT0 = qtp.tile([P, ND, P], f16, tag="QT", name="QT0_0pe")
        pstq = psumT.tile([P, ND, P], f16, tag="pst", name="pstq")
        for k in range(ND):
            chain(
                nc.tensor.transpose(
                    pstq[:, k, :], Qn0[:, k * P : (k + 1) * P], ident
                )
            )
        nc.vector.tensor_copy(QT0, pstq)

        QT_next = QT0
        nxt_v = None
        pending = None
        fin = None
        NS = BPC * NQT

        def b0_prologue(n):
            # interleave batch-0 V transposes with the first tile's chunks
            for j in range(4 * n, 4 * n + 4):
                trans_v(0, *cur_v, j)

        for s in range(NS):
            b, qi = divmod(s, NQT)
            Vn, VT = cur_v
            QT = QT_next
            if fin is not None:
                tail_fin(*fin)
                fin = None
            if b + 1 < BPC:
                if qi == 1:
                    nxt_v = alloc_v(b + 1)
                if 8 <= qi < 16:
                    # Natural V for the next batch, 2 v-tiles per call —
                    # emitted before the QT XBAR so it shares the store's
                    # regular-DMA epoch (XBARs serialize against regular
                    # DMAs in scheduled order).
                    load_vn_pair(b + 1, nxt_v[0], (qi - 8) * 2)
            if s + 1 < NS:
                QT_next = issue_qt(*divmod(s + 1, NQT))
            if b + 1 < BPC and 1 <= qi < 9:
                # VT for the next batch via XBAR, 2 v-tiles per call,
                # adjacent to the QT XBAR (no epoch handoff between).
                issue_vt_xbar(b + 1, nxt_v[1], (qi - 1) * 2)
            psS, stats = head(b, qi, VT, QT, prologue=b0_prologue if s == 0 else None)
            PT, rinv = softmax(b, qi, psS, stats)
            if pending is not None:
                pb, pq, pPT, prinv, pVn = pending
                psO = tail_mm(pb, pq, pPT, prinv, pVn)
                fin = (pb, pq, psO, prinv)
            pending = (b, qi, PT, rinv, Vn)
            if qi == NQT - 1 and b + 1 < BPC:
                cur_v = nxt_v
        if fin is not None:
            tail_fin(*fin)
        pb, pq, pPT, prinv, pVn = pending
        psO = tail_mm(pb, pq, pPT, prinv, pVn)
        tail_fin(pb, pq, psO, prinv)

    nc.compile()
    return nc


_NC_CACHE = None


def _get_nc():
    global _NC_CACHE
    if _NC_CACHE is None:
        _NC_CACHE = build_nc()
    return _NC_CACHE


def kernel(query: np.ndarray, value: np.ndarray) -> np.ndarray:
    query = np.asarray(query)
    value = np.asarray(value)
    assert query.shape == (B, LQ, D) and value.shape == (B, LV, D)
    q16 = np.ascontiguousarray(query.astype(np.float16))
    v16 = np.ascontiguousarray(value.astype(np.float16))
    nc = _get_nc()
    in_maps = [
        {
            "q16": q16[i * BPC : (i + 1) * BPC],
            "v16": v16[i * BPC : (i + 1) * BPC],
        }
        for i in range(NCORES)
    ]
    res = run_bass_kernel_spmd(nc, in_maps, list(range(NCORES)))
    out = np.concatenate(
        [res.results[i]["o"].astype(np.float32) for i in range(NCORES)], axis=0
    )
    return out


# revision 9
# speedup vs baseline: 1.0683x; 1.0330x over previous
"""TRN2 Bass kernel for batched dot-product attention (no scale, eval mode).

reference:
    score   = einsum('bqd,bvd->bqv', query, value)      # B=16, L=2048, D=1024
    attn    = softmax(score, axis=-1)
    context = einsum('bqv,bvd->bqd', attn, value)

Sharding: data-parallel over batch; each of 8 NeuronCores handles 2 batch
elements, no communication. Inputs are pre-cast to fp16 on the host; matmuls
run fp16 with fp32 PSUM accumulation.

Key structure (v2 — startup/tail/transpose-offload rework of the v1 design):
  - All XBAR (dma_start_transpose) calls issue from the SP (sync) engine
    only: the XBAR completion semaphores (DMAHW ring) are shared round-robin
    across issuing engines, so transposes from two engines race the ring and
    consumers can observe stale tiles (hard-won HW lesson from v1).
  - Tile serializes XBAR transposes against regular DMAs in epochs: each
    XBAR waits for every previously-scheduled regular DMA and vice versa.
    The per-step DMA chain is therefore budgeted: [store+loads] -> QT xbar
    -> (VT xbar) -> PT xbar x2, which fits inside the 13.7us PE step.
  - Batch-0 V transposes run on the PE (identity matmul + DVE copy out of
    PSUM), interleaved with the first tile's MM1 chunks: V arrives at HBM
    rate (~11us) and the PE does useful transpose work while waiting, which
    also warms the HAM clock gate.
  - Batch-1 V transposes instead go through the XBAR straight from DRAM
    (no Vn dependency), dripped 2 v-tiles per step during batch 0: this
    removes 128 PE transposes (~14us of PE time) from the steady state.
  - Batch-0 Vn loads issue from SP (HWDGE) — SWDGE (gpsimd) adds ~5.5us of
    ring latency which sat on the v1 critical path at startup. Batch-1 Vn
    loads stay on gpsimd (huge slack), 2 v-tiles per call, scheduled in the
    same DMA epoch as the output store of the step.
  - Q-tile-0's transpose runs on the PE from a natural Qn load, so the
    first MM1 chunk is not gated on any XBAR call at startup.
  - PE instructions are explicitly chained in emission order (sync=False
    dep edges): the tile scheduler otherwise reorders PE instructions and
    can split PSUM accumulation groups, which corrupts results on HW
    (CoreSim's per-address PSUM model tolerates it, hardware does not).
  - MM1 writes four separate 1-bank PSUM chunk tiles (not one 4-bank tile)
    so the WAR edges (rowmax/exp of tile s vs MM1 of tile s+1) resolve per
    chunk and never stall the PE.
  - MM2 writes two separate 1-bank psO chunk tiles; the two normalization
    multiplies are split across engines (dch0 on DVE, dch1 on ACT) and
    EMITTED ONE STEP AFTER their MM2 (at the head of the next step's engine
    programs): when they queue behind the same step's rowmax/exp chains,
    the psO WAR release throttles the next tile's MM2 into a ~1us/step
    stall. Per-chunk psO lets the final tile's dch0 normalization overlap
    its dch1 matmuls instead of waiting for all of them.
  - Output stores run on ACT (HWDGE): the v1 gpsimd (SWDGE) stores cost a
    ~6.4us ring-drain after the last tile.
  - rowsum comes from a DVE reduce over the fp16 P tile (no ACT accum_out):
    fewer ACT instructions and the psS chunks' reader set stays {max, exp}.

Per-core per-batch plan:
  - batch 0: Qn0 + Vn natural fp16 via SP HWDGE; QT0 and VT via PE
    transposes (VT interleaved with the first tile's MM1 chunks)
  - batch 1: VT via XBAR from DRAM (2 v-tiles/step, qi 1..8 of batch 0),
    Vn natural via gpsimd (2 v-tiles/call, qi 8..15 of batch 0)
  - per 128-row q-tile (1-deep software pipeline; MM2 lags one tile):
      QT via XBAR from DRAM (sync queue, issued one step ahead)
      MM1: S = QT.T @ VT -> 4x512 PSUM chunks, per-chunk rowmax on DVE
      softmax: exp(S - max) on ACT (fp16 P, fused rowsum), then PT via XBAR
      MM2: O = PT.T @ Vn -> 2x512 PSUM chunks, scale by 1/rowsum, store
"""

from contextlib import ExitStack

import numpy as np

import concourse.tile as tile
from concourse import bacc, mybir
from concourse.bass import _add_dep_helper
from concourse.masks import make_identity
from concourse.bass_utils import run_bass_kernel_spmd

B, LQ, LV, D = 16, 2048, 2048, 1024
NCORES = 8
BPC = B // NCORES  # batches per core
P = 128
NQT = LQ // P  # 16 q tiles
NVT = LV // P  # 16 v tiles
ND = D // P  # 8 d tiles
VCH = 512  # MM1 matmul group width (one bank of f32)
SCH = 1024  # softmax chunk width (one 2-bank psS tile)
NCH = LV // SCH  # 2
DCH = 512  # MM2 PSUM chunk
NDCH = D // DCH  # 2

f32 = mybir.dt.float32
f16 = mybir.dt.float16
EXP = mybir.ActivationFunctionType.Exp
AX = mybir.AxisListType.X


def build_nc():
    nc = bacc.Bacc("TRN2", target_bir_lowering=False, debug=False)
    q_d = nc.dram_tensor("q16", [BPC, LQ, D], f16, kind="ExternalInput").ap()
    v_d = nc.dram_tensor("v16", [BPC, LV, D], f16, kind="ExternalInput").ap()
    o_d = nc.dram_tensor("o", [BPC, LQ, D], f16, kind="ExternalOutput").ap()

    prev_pe = [None]

    def chain(inst):
        """Order every PE instruction after the previous one (scheduler-order
        edge only; same-engine, so no runtime semaphore is needed)."""
        if prev_pe[0] is not None:
            _add_dep_helper(inst.ins, prev_pe[0].ins, sync=False, reason="pe-order")
        prev_pe[0] = inst
        return inst

    prev_sp = [None]

    def schain(inst):
        """Order every SP (sync) DMA/XBAR instruction in emission order.
        The tile scheduler otherwise hoists dep-free instructions (batch-1
        VT XBARs, prefetch loads) to the front of the sync FIFO, where they
        delay the PT transposes of the early steps and starve the batch-0
        V feed at startup (observed: 4.5us PE gap + HAM re-throttle)."""
        if prev_sp[0] is not None:
            _add_dep_helper(inst.ins, prev_sp[0].ins, sync=False, reason="sp-order")
        prev_sp[0] = inst
        return inst

    with tile.TileContext(nc) as tc, ExitStack() as ctx:
        const = ctx.enter_context(tc.tile_pool(name="const", bufs=1))
        vtp = ctx.enter_context(tc.tile_pool(name="vtp", bufs=2))
        vnp = ctx.enter_context(tc.tile_pool(name="vnp", bufs=2))
        qtp = ctx.enter_context(tc.tile_pool(name="qtp", bufs=4))
        qnp = ctx.enter_context(tc.tile_pool(name="qnp", bufs=1))
        pp = ctx.enter_context(tc.tile_pool(name="pp", bufs=2))
        ptp = ctx.enter_context(tc.tile_pool(name="ptp", bufs=2))
        outp = ctx.enter_context(tc.tile_pool(name="outp", bufs=2))
        statp = ctx.enter_context(tc.tile_pool(name="statp", bufs=3))
        psumS = ctx.enter_context(tc.tile_pool(name="psumS", bufs=1, space="PSUM"))
        psumO = ctx.enter_context(tc.tile_pool(name="psumO", bufs=1, space="PSUM"))
        psumT = ctx.enter_context(tc.tile_pool(name="psumT", bufs=2, space="PSUM"))

        ident = const.tile([P, P], f16)
        make_identity(nc, ident)

        # Warm the ACT exp table: the first EXP activation triggers a ~1.3us
        # ACT_TABLE_LOAD which otherwise lands on the step-0 softmax critical
        # path. ACT is idle at startup, so this is free.
        warm = statp.tile([P, 1], f32, tag="warm", name="warm_exp")
        nc.scalar.activation(warm, ident[:, 0:1], EXP)

        def alloc_v(b):
            Vn = vnp.tile([P, NVT, D], f16, tag="Vn", name=f"Vn{b}")
            VT = vtp.tile([P, ND, LV], f16, tag="VT", name=f"VT{b}")
            return Vn, VT

        def trans_v(b, Vn, VT, j):
            """VT[d, k, j*P+v'] = V[j*P+v', k*P+d] via PE transpose + DVE copy."""
            pst = psumT.tile([P, ND, P], f16, tag="pst", name=f"pst{b}_{j}")
            for k in range(ND):
                chain(
                    nc.tensor.transpose(
                        pst[:, k, :], Vn[:, j, k * P : (k + 1) * P], ident
                    )
                )
            nc.vector.tensor_copy(VT[:, :, j * P : (j + 1) * P], pst)

        def issue_qt(b, qi):
            """QT[d', k, q] = Q[qi*P+q, k*P+d'] via XBAR from DRAM (sync)."""
            QT = qtp.tile([P, ND, P], f16, tag="QT", name=f"QT{b}_{qi}")
            schain(nc.sync.dma_start_transpose(QT, q_d[b, qi * P : (qi + 1) * P, :]))
            return QT

        def issue_vt_xbar(b, VT, j):
            """VT[:, :, j*P:(j+2)*P] for two v-tiles via XBAR from DRAM."""
            schain(
                nc.sync.dma_start_transpose(
                    VT[:, :, j * P : (j + 2) * P],
                    v_d[b, j * P : (j + 2) * P, :],
                )
            )

        def load_vn_pair(b, Vn, j):
            """Natural-layout V for the next batch, two v-tiles per call, on
            the sync queue so schain pins it into its step (the scheduler
            otherwise hoists it to t=0 where it competes with the batch-0
            V feed for HBM bandwidth)."""
            schain(
                nc.sync.dma_start(
                    out=Vn[:, j : j + 2, :],
                    in_=v_d[b, j * P : (j + 2) * P, :].rearrange(
                        "(t p) d -> p t d", p=P
                    ),
                )
            )

        def head(b, qi, VT, QT, prologue=None):
            """S = Q @ V^T into two 2-bank PSUM chunks; per-chunk row maxes.
            Matmul accumulation groups stay one bank (512 f32) wide."""
            psS = []
            stats = statp.tile([P, NCH], f32, tag="stats", name=f"st{b}_{qi}")
            for n in range(NCH):
                ps = psumS.tile([P, SCH], f32, tag=f"psS{n}", name=f"psS{b}_{qi}_{n}")
                psS.append(ps)
                for h2 in range(2):
                    if prologue is not None:
                        prologue(2 * n + h2)
                    v0 = n * SCH + h2 * VCH
                    for k in range(ND):
                        chain(
                            nc.tensor.matmul(
                                ps[:, h2 * VCH : (h2 + 1) * VCH],
                                QT[:, k, :],
                                VT[:, k, v0 : v0 + VCH],
                                start=(k == 0),
                                stop=(k == ND - 1),
                            )
                        )
                nc.vector.reduce_max(stats[:, n : n + 1], ps, axis=AX)
            return psS, stats

        def softmax(b, qi, psS, stats):
            """exp(S - max) -> fp16 P (fused rowsum); PT via XBAR; 1/rowsum."""
            negmax = statp.tile([P, 1], f32, tag="negmax", name=f"nm{b}_{qi}")
            nc.vector.reduce_max(negmax, stats, axis=AX, negate=True)
            Pt = pp.tile([P, LV], f16, tag="P", name=f"P{b}_{qi}")
            PT = ptp.tile([P, NVT, P], f16, tag="PT", name=f"PT{b}_{qi}")
            for n in range(NCH):
                nc.scalar.activation(
                    Pt[:, n * SCH : (n + 1) * SCH],
                    psS[n],
                    EXP,
                    bias=negmax,
                )
            for h in range(2):
                # PT[v', j, q] = P[q, j*P+v'], one 1024-wide half per call,
                # on sync (the only XBAR engine), gated on exp chunk h.
                schain(
                    nc.sync.dma_start_transpose(
                        PT[:, h * ND : (h + 1) * ND, :],
                        Pt[:, h * SCH : (h + 1) * SCH],
                    )
                )
            rowsum = statp.tile([P, 1], f32, tag="rowsum", name=f"rs{b}_{qi}")
            nc.vector.reduce_sum(rowsum, Pt, axis=AX)
            rinv = statp.tile([P, 1], f32, tag="rinv", name=f"ri{b}_{qi}")
            nc.vector.reciprocal(rinv, rowsum)
            return PT, rinv

        def tail_mm(b, qi, PT, rinv, Vn):
            """O = P @ V into two 1-bank PSUM chunks; normalize/store deferred
            one step. Per-chunk tiles let WAR edges resolve per chunk."""
            psO = []
            for dch in range(NDCH):
                ps = psumO.tile(
                    [P, DCH], f32, tag=f"psO{dch}", name=f"psO{b}_{qi}_{dch}"
                )
                psO.append(ps)
                sl = slice(dch * DCH, (dch + 1) * DCH)
                for j in range(NVT):
                    chain(
                        nc.tensor.matmul(
                            ps,
                            PT[:, j, :],
                            Vn[:, j, sl],
                            start=(j == 0),
                            stop=(j == NVT - 1),
                        )
                    )
            return psO

        def tail_fin(b, qi, psO, rinv):
            """Normalize to fp16 (dch0 on DVE, dch1 on ACT), store (ACT HWDGE).
            Emitted at the head of the NEXT step's engine programs so the
            muls run the moment MM2 finishes, releasing psO early."""
            out16 = outp.tile([P, D], f16, tag="out", name=f"o{b}_{qi}")
            nc.vector.tensor_scalar_mul(out16[:, :DCH], psO[0], rinv)
            nc.scalar.mul(out16[:, DCH:], psO[1], rinv)
            nc.scalar.dma_start(out=o_d[b, qi * P : (qi + 1) * P, :], in_=out16)

        # --- batch-0 startup -------------------------------------------------
        cur_v = alloc_v(0)
        # Qn tile 0 natural + batch-0 Vn, all on SP HWDGE (low latency).
        Qn0 = qnp.tile([P, D], f16, tag="Qn0", name="Qn0")
        schain(nc.sync.dma_start(out=Qn0, in_=q_d[0, 0:P, :]))
        for j in range(NVT):
            schain(
                nc.sync.dma_start(
                    out=cur_v[0][:, j, :], in_=v_d[0, j * P : (j + 1) * P, :]
                )
            )
        # QT for (0, 0) via PE transpose from Qn0 — no XBAR on the critical
        # path at startup.
        QT0 = qtp.tile([P, ND, P], f16, tag="QT", name="QT0_0pe")
        pstq = psumT.tile([P, ND, P], f16, tag="pst", name="pstq")
        for k in range(ND):
            chain(
                nc.tensor.transpose(
                    pstq[:, k, :], Qn0[:, k * P : (k + 1) * P], ident
                )
            )
        nc.vector.tensor_copy(QT0, pstq)

        QT_next = QT0
        nxt_v = None
        pending = None
        fin = None
        NS = BPC * NQT

        # Transpose v-tile group 0 before the first MM1 chunk; the prologue
        # then keeps a one-group lead (group m+1 before chunk-half m) so the
        # PSUM->SBUF copies of a group finish during the previous half's MMs
        # instead of sitting on the MM1 critical path.
        for j in range(4):
            trans_v(0, *cur_v, j)

        def b0_prologue(m):
            # interleave batch-0 V transposes with the first tile's chunks
            if m < 3:
                for j in range(4 * (m + 1), 4 * (m + 1) + 4):
                    trans_v(0, *cur_v, j)

        for s in range(NS):
            b, qi = divmod(s, NQT)
            Vn, VT = cur_v
            QT = QT_next
            if fin is not None:
                tail_fin(*fin)
                fin = None
            if b + 1 < BPC:
                if qi == 1:
                    nxt_v = alloc_v(b + 1)
                if 8 <= qi < 16:
                    # Natural V for the next batch, 2 v-tiles per call —
                    # emitted before the QT XBAR so it shares the store's
                    # regular-DMA epoch (XBARs serialize against regular
                    # DMAs in scheduled order).
                    load_vn_pair(b + 1, nxt_v[0], (qi - 8) * 2)
            if s + 1 < NS:
                QT_next = issue_qt(*divmod(s + 1, NQT))
            if b + 1 < BPC and 1 <= qi < 9:
                # VT for the next batch via XBAR, 2 v-tiles per call,
                # adjacent to the QT XBAR (no epoch handoff between).
                issue_vt_xbar(b + 1, nxt_v[1], (qi - 1) * 2)
            psS, stats = head(b, qi, VT, QT, prologue=b0_prologue if s == 0 else None)
            PT, rinv = softmax(b, qi, psS, stats)
            if pending is not None:
                pb, pq, pPT, prinv, pVn = pending
                psO = tail_mm(pb, pq, pPT, prinv, pVn)
                fin = (pb, pq, psO, prinv)
            pending = (b, qi, PT, rinv, Vn)
            if qi == NQT - 1 and b + 1 < BPC:
                cur_v = nxt_v
        if fin is not None:
            tail_fin(*fin)
        pb, pq, pPT, prinv, pVn = pending
        psO = tail_mm(pb, pq, pPT, prinv, pVn)
        tail_fin(pb, pq, psO, prinv)

    nc.compile()
    return nc


_NC_CACHE = None


def _get_nc():
    global _NC_CACHE
    if _NC_CACHE is None:
        _NC_CACHE = build_nc()
    return _NC_CACHE


def kernel(query: np.ndarray, value: np.ndarray) -> np.ndarray:
    query = np.asarray(query)
    value = np.asarray(value)
    assert query.shape == (B, LQ, D) and value.shape == (B, LV, D)
    q16 = np.ascontiguousarray(query.astype(np.float16))
    v16 = np.ascontiguousarray(value.astype(np.float16))
    nc = _get_nc()
    in_maps = [
        {
            "q16": q16[i * BPC : (i + 1) * BPC],
            "v16": v16[i * BPC : (i + 1) * BPC],
        }
        for i in range(NCORES)
    ]
    res = run_bass_kernel_spmd(nc, in_maps, list(range(NCORES)))
    out = np.concatenate(
        [res.results[i]["o"].astype(np.float32) for i in range(NCORES)], axis=0
    )
    return out


# revision 17
# speedup vs baseline: 1.0752x; 1.0065x over previous
"""TRN2 Bass kernel for batched dot-product attention (no scale, eval mode).

reference:
    score   = einsum('bqd,bvd->bqv', query, value)      # B=16, L=2048, D=1024
    attn    = softmax(score, axis=-1)
    context = einsum('bqv,bvd->bqd', attn, value)

Sharding: data-parallel over batch; each of 8 NeuronCores handles 2 batch
elements, no communication. Inputs are pre-cast to fp16 on the host; matmuls
run fp16 with fp32 PSUM accumulation.

Key structure (v2 — startup/tail/transpose-offload rework of the v1 design):
  - All XBAR (dma_start_transpose) calls issue from the SP (sync) engine
    only: the XBAR completion semaphores (DMAHW ring) are shared round-robin
    across issuing engines, so transposes from two engines race the ring and
    consumers can observe stale tiles (hard-won HW lesson from v1).
  - Tile serializes XBAR transposes against regular DMAs in epochs: each
    XBAR waits for every previously-scheduled regular DMA and vice versa.
    The per-step DMA chain is therefore budgeted: [store+loads] -> QT xbar
    -> (VT xbar) -> PT xbar x2, which fits inside the 13.7us PE step.
  - Batch-0 V transposes run on the PE (identity matmul + DVE copy out of
    PSUM), interleaved with the first tile's MM1 chunks: V arrives at HBM
    rate (~11us) and the PE does useful transpose work while waiting, which
    also warms the HAM clock gate.
  - Batch-1 V transposes instead go through the XBAR straight from DRAM
    (no Vn dependency), dripped 2 v-tiles per step during batch 0: this
    removes 128 PE transposes (~14us of PE time) from the steady state.
  - Batch-0 Vn loads issue from SP (HWDGE) — SWDGE (gpsimd) adds ~5.5us of
    ring latency which sat on the v1 critical path at startup. Batch-1 Vn
    loads stay on gpsimd (huge slack), 2 v-tiles per call, scheduled in the
    same DMA epoch as the output store of the step.
  - Q-tile-0's transpose runs on the PE from a natural Qn load, so the
    first MM1 chunk is not gated on any XBAR call at startup.
  - PE instructions are explicitly chained in emission order (sync=False
    dep edges): the tile scheduler otherwise reorders PE instructions and
    can split PSUM accumulation groups, which corrupts results on HW
    (CoreSim's per-address PSUM model tolerates it, hardware does not).
  - MM1 writes four separate 1-bank PSUM chunk tiles (not one 4-bank tile)
    so the WAR edges (rowmax/exp of tile s vs MM1 of tile s+1) resolve per
    chunk and never stall the PE.
  - MM2 writes two separate 1-bank psO chunk tiles; the two normalization
    multiplies are split across engines (dch0 on DVE, dch1 on ACT) and
    EMITTED ONE STEP AFTER their MM2 (at the head of the next step's engine
    programs): when they queue behind the same step's rowmax/exp chains,
    the psO WAR release throttles the next tile's MM2 into a ~1us/step
    stall. Per-chunk psO lets the final tile's dch0 normalization overlap
    its dch1 matmuls instead of waiting for all of them.
  - Output stores run on ACT (HWDGE): the v1 gpsimd (SWDGE) stores cost a
    ~6.4us ring-drain after the last tile.
  - rowsum comes from a DVE reduce over the fp16 P tile (no ACT accum_out):
    fewer ACT instructions and the psS chunks' reader set stays {max, exp}.

Per-core per-batch plan:
  - batch 0: Qn0 + Vn natural fp16 via SP HWDGE; QT0 and VT via PE
    transposes (VT interleaved with the first tile's MM1 chunks)
  - batch 1: VT via XBAR from DRAM (2 v-tiles/step, qi 1..8 of batch 0),
    Vn natural via gpsimd (2 v-tiles/call, qi 8..15 of batch 0)
  - per 128-row q-tile (1-deep software pipeline; MM2 lags one tile):
      QT via XBAR from DRAM (sync queue, issued one step ahead)
      MM1: S = QT.T @ VT -> 4x512 PSUM chunks, per-chunk rowmax on DVE
      softmax: exp(S - max) on ACT (fp16 P, fused rowsum), then PT via XBAR
      MM2: O = PT.T @ Vn -> 2x512 PSUM chunks, scale by 1/rowsum, store
"""

from contextlib import ExitStack

import numpy as np

import concourse.tile as tile
from concourse import bacc, mybir
from concourse.bass import _add_dep_helper
from concourse.masks import make_identity
from concourse.bass_utils import run_bass_kernel_spmd

B, LQ, LV, D = 16, 2048, 2048, 1024
NCORES = 8
BPC = B // NCORES  # batches per core
P = 128
NQT = LQ // P  # 16 q tiles
NVT = LV // P  # 16 v tiles
ND = D // P  # 8 d tiles
VCH = 512  # MM1 matmul group width (one bank of f32)
SCH = 1024  # softmax chunk width (one 2-bank psS tile)
NCH = LV // SCH  # 2
DCH = 512  # MM2 PSUM chunk
NDCH = D // DCH  # 2

f32 = mybir.dt.float32
f16 = mybir.dt.float16
EXP = mybir.ActivationFunctionType.Exp
AX = mybir.AxisListType.X


def build_nc():
    nc = bacc.Bacc("TRN2", target_bir_lowering=False, debug=False)
    q_d = nc.dram_tensor("q16", [BPC, LQ, D], f16, kind="ExternalInput").ap()
    v_d = nc.dram_tensor("v16", [BPC, LV, D], f16, kind="ExternalInput").ap()
    o_d = nc.dram_tensor("o", [BPC, LQ, D], f16, kind="ExternalOutput").ap()

    prev_pe = [None]

    def chain(inst):
        """Order every PE instruction after the previous one (scheduler-order
        edge only; same-engine, so no runtime semaphore is needed)."""
        if prev_pe[0] is not None:
            _add_dep_helper(inst.ins, prev_pe[0].ins, sync=False, reason="pe-order")
        prev_pe[0] = inst
        return inst

    prev_sp = [None]

    def schain(inst):
        """Order every SP (sync) DMA/XBAR instruction in emission order.
        The tile scheduler otherwise hoists dep-free instructions (batch-1
        VT XBARs, prefetch loads) to the front of the sync FIFO, where they
        delay the PT transposes of the early steps and starve the batch-0
        V feed at startup (observed: 4.5us PE gap + HAM re-throttle)."""
        if prev_sp[0] is not None:
            _add_dep_helper(inst.ins, prev_sp[0].ins, sync=False, reason="sp-order")
        prev_sp[0] = inst
        return inst

    with tile.TileContext(nc) as tc, ExitStack() as ctx:
        const = ctx.enter_context(tc.tile_pool(name="const", bufs=1))
        vtp = ctx.enter_context(tc.tile_pool(name="vtp", bufs=2))
        vnp = ctx.enter_context(tc.tile_pool(name="vnp", bufs=2))
        qtp = ctx.enter_context(tc.tile_pool(name="qtp", bufs=4))
        qnp = ctx.enter_context(tc.tile_pool(name="qnp", bufs=1))
        pp = ctx.enter_context(tc.tile_pool(name="pp", bufs=2))
        ptp = ctx.enter_context(tc.tile_pool(name="ptp", bufs=2))
        outp = ctx.enter_context(tc.tile_pool(name="outp", bufs=2))
        statp = ctx.enter_context(tc.tile_pool(name="statp", bufs=3))
        psumS = ctx.enter_context(tc.tile_pool(name="psumS", bufs=1, space="PSUM"))
        psumO = ctx.enter_context(tc.tile_pool(name="psumO", bufs=1, space="PSUM"))
        psumT = ctx.enter_context(tc.tile_pool(name="psumT", bufs=2, space="PSUM"))

        ident = const.tile([P, P], f16)
        make_identity(nc, ident)

        # Warm the ACT exp table: the first EXP activation triggers a ~1.3us
        # ACT_TABLE_LOAD which otherwise lands on the step-0 softmax critical
        # path. ACT is idle at startup, so this is free.
        warm = statp.tile([P, 1], f32, tag="warm", name="warm_exp")
        nc.scalar.activation(warm, ident[:, 0:1], EXP)

        def alloc_v(b):
            Vn = vnp.tile([P, NVT, D], f16, tag="Vn", name=f"Vn{b}")
            VT = vtp.tile([P, ND, LV], f16, tag="VT", name=f"VT{b}")
            return Vn, VT

        def trans_v(b, Vn, VT, j):
            """VT[d, k, j*P+v'] = V[j*P+v', k*P+d] via PE transpose + DVE copy."""
            pst = psumT.tile([P, ND, P], f16, tag="pst", name=f"pst{b}_{j}")
            for k in range(ND):
                chain(
                    nc.tensor.transpose(
                        pst[:, k, :], Vn[:, j, k * P : (k + 1) * P], ident
                    )
                )
            nc.vector.tensor_copy(VT[:, :, j * P : (j + 1) * P], pst)

        def issue_qt(b, qi):
            """QT[d', k, q] = Q[qi*P+q, k*P+d'] via XBAR from DRAM (sync)."""
            QT = qtp.tile([P, ND, P], f16, tag="QT", name=f"QT{b}_{qi}")
            schain(nc.sync.dma_start_transpose(QT, q_d[b, qi * P : (qi + 1) * P, :]))
            return QT

        def issue_vt_xbar(b, VT, j):
            """VT[:, :, j*P:(j+2)*P] for two v-tiles via XBAR from DRAM."""
            schain(
                nc.sync.dma_start_transpose(
                    VT[:, :, j * P : (j + 2) * P],
                    v_d[b, j * P : (j + 2) * P, :],
                )
            )

        def load_vn_pair(b, Vn, j):
            """Natural-layout V for the next batch, two v-tiles per call, on
            the sync queue so schain pins it into its step (the scheduler
            otherwise hoists it to t=0 where it competes with the batch-0
            V feed for HBM bandwidth)."""
            schain(
                nc.sync.dma_start(
                    out=Vn[:, j : j + 2, :],
                    in_=v_d[b, j * P : (j + 2) * P, :].rearrange(
                        "(t p) d -> p t d", p=P
                    ),
                )
            )

        def head(b, qi, VT, QT, prologue=None):
            """S = Q @ V^T into two 2-bank PSUM chunks; per-chunk row maxes.
            Matmul accumulation groups stay one bank (512 f32) wide. (A
            per-512-half rowmax variant serializes against the next half's
            MMs in the tile tracker and costs ~55us — keep per-chunk.)"""
            psS = []
            stats = statp.tile([P, NCH], f32, tag="stats", name=f"st{b}_{qi}")
            for n in range(NCH):
                ps = psumS.tile([P, SCH], f32, tag=f"psS{n}", name=f"psS{b}_{qi}_{n}")
                psS.append(ps)
                for h2 in range(2):
                    if prologue is not None:
                        prologue(2 * n + h2)
                    v0 = n * SCH + h2 * VCH
                    for k in range(ND):
                        chain(
                            nc.tensor.matmul(
                                ps[:, h2 * VCH : (h2 + 1) * VCH],
                                QT[:, k, :],
                                VT[:, k, v0 : v0 + VCH],
                                start=(k == 0),
                                stop=(k == ND - 1),
                            )
                        )
                nc.vector.reduce_max(stats[:, n : n + 1], ps, axis=AX)
            return psS, stats

        def softmax(b, qi, psS, stats):
            """exp(S - max) -> fp16 P (fused rowsum); PT via XBAR; 1/rowsum."""
            negmax = statp.tile([P, 1], f32, tag="negmax", name=f"nm{b}_{qi}")
            nc.vector.reduce_max(negmax, stats, axis=AX, negate=True)
            Pt = pp.tile([P, LV], f16, tag="P", name=f"P{b}_{qi}")
            PT = ptp.tile([P, NVT, P], f16, tag="PT", name=f"PT{b}_{qi}")
            for n in range(NCH):
                nc.scalar.activation(
                    Pt[:, n * SCH : (n + 1) * SCH],
                    psS[n],
                    EXP,
                    bias=negmax,
                )
            for h in range(2):
                # PT[v', j, q] = P[q, j*P+v'], one 1024-wide half per call,
                # on sync (the only XBAR engine), gated on exp chunk h.
                schain(
                    nc.sync.dma_start_transpose(
                        PT[:, h * ND : (h + 1) * ND, :],
                        Pt[:, h * SCH : (h + 1) * SCH],
                    )
                )
            rowsum = statp.tile([P, 1], f32, tag="rowsum", name=f"rs{b}_{qi}")
            nc.vector.reduce_sum(rowsum, Pt, axis=AX)
            rinv = statp.tile([P, 1], f32, tag="rinv", name=f"ri{b}_{qi}")
            nc.vector.reciprocal(rinv, rowsum)
            return PT, rinv

        def tail_mm(b, qi, PT, rinv, Vn):
            """O = P @ V into two 1-bank PSUM chunks; normalize/store deferred
            one step. Per-chunk tiles let WAR edges resolve per chunk."""
            psO = []
            for dch in range(NDCH):
                ps = psumO.tile(
                    [P, DCH], f32, tag=f"psO{dch}", name=f"psO{b}_{qi}_{dch}"
                )
                psO.append(ps)
                sl = slice(dch * DCH, (dch + 1) * DCH)
                for j in range(NVT):
                    chain(
                        nc.tensor.matmul(
                            ps,
                            PT[:, j, :],
                            Vn[:, j, sl],
                            start=(j == 0),
                            stop=(j == NVT - 1),
                        )
                    )
            return psO

        def tail_fin(b, qi, psO, rinv, last=False):
            """Normalize to fp16 (dch0 on DVE, dch1 on ACT), then store.
            Emitted at the head of the NEXT step's engine programs so the
            muls run the moment MM2 finishes, releasing psO early. Stores go
            on gpsimd (idle engine; an ACT store gets epoch-blocked behind
            the PT XBARs and then delays the following exp in the ACT FIFO,
            stalling the next tile's MM1 on the psS WAR release) — except the
            final two tiles, where ACT's low HWDGE latency avoids the ~6us
            SWDGE ring-drain at kernel exit."""
            out16 = outp.tile([P, D], f16, tag="out", name=f"o{b}_{qi}")
            nc.vector.tensor_scalar_mul(out16[:, :DCH], psO[0], rinv)
            nc.scalar.mul(out16[:, DCH:], psO[1], rinv)
            eng = nc.scalar if last else nc.gpsimd
            eng.dma_start(out=o_d[b, qi * P : (qi + 1) * P, :], in_=out16)

        # --- batch-0 startup -------------------------------------------------
        cur_v = alloc_v(0)
        # Qn tile 0 natural + batch-0 Vn, all on SP HWDGE (low latency).
        # Vn j0 goes first: the first V transpose needs it right after the
        # Q-transposes, while Qn0 is only needed once the identity is ready.
        Qn0 = qnp.tile([P, D], f16, tag="Qn0", name="Qn0")
        schain(
            nc.sync.dma_start(out=cur_v[0][:, 0, :], in_=v_d[0, 0:P, :])
        )
        schain(nc.sync.dma_start(out=Qn0, in_=q_d[0, 0:P, :]))
        for j in range(1, NVT):
            schain(
                nc.sync.dma_start(
                    out=cur_v[0][:, j, :], in_=v_d[0, j * P : (j + 1) * P, :]
                )
            )
        # QT for (0, 0) via PE transpose from Qn0 — no XBAR on the critical
        # path at startup.
        QT0 = qtp.tile([P, ND, P], f16, tag="QT", name="QT0_0pe")
        pstq = psumT.tile([P, ND, P], f16, tag="pst", name="pstq")
        for k in range(ND):
            chain(
                nc.tensor.transpose(
                    pstq[:, k, :], Qn0[:, k * P : (k + 1) * P], ident
                )
            )
        nc.vector.tensor_copy(QT0, pstq)

        QT_next = QT0
        nxt_v = None
        pending = None
        fin = None
        NS = BPC * NQT

        # Transpose v-tile group 0 before the first MM1 chunk; the prologue
        # then keeps a one-group lead (group m+1 before chunk-half m) so the
        # PSUM->SBUF copies of a group finish during the previous half's MMs
        # instead of sitting on the MM1 critical path.
        for j in range(4):
            trans_v(0, *cur_v, j)

        def b0_prologue(m):
            # interleave batch-0 V transposes with the first tile's chunks
            if m < 3:
                for j in range(4 * (m + 1), 4 * (m + 1) + 4):
                    trans_v(0, *cur_v, j)

        for s in range(NS):
            b, qi = divmod(s, NQT)
            Vn, VT = cur_v
            QT = QT_next
            if fin is not None:
                tail_fin(*fin)
                fin = None
            if b + 1 < BPC:
                if qi == 1:
                    nxt_v = alloc_v(b + 1)
                if 8 <= qi < 16:
                    # Natural V for the next batch, 2 v-tiles per call —
                    # emitted before the QT XBAR so it shares the store's
                    # regular-DMA epoch (XBARs serialize against regular
                    # DMAs in scheduled order).
                    load_vn_pair(b + 1, nxt_v[0], (qi - 8) * 2)
            if s + 1 < NS:
                QT_next = issue_qt(*divmod(s + 1, NQT))
            if b + 1 < BPC and 1 <= qi < 9:
                # VT for the next batch via XBAR, 2 v-tiles per call,
                # adjacent to the QT XBAR (no epoch handoff between).
                issue_vt_xbar(b + 1, nxt_v[1], (qi - 1) * 2)
            psS, stats = head(b, qi, VT, QT, prologue=b0_prologue if s == 0 else None)
            PT, rinv = softmax(b, qi, psS, stats)
            if pending is not None:
                pb, pq, pPT, prinv, pVn = pending
                psO = tail_mm(pb, pq, pPT, prinv, pVn)
                fin = (pb, pq, psO, prinv)
            pending = (b, qi, PT, rinv, Vn)
            if qi == NQT - 1 and b + 1 < BPC:
                cur_v = nxt_v
        if fin is not None:
            tail_fin(*fin, last=True)
        pb, pq, pPT, prinv, pVn = pending
        psO = tail_mm(pb, pq, pPT, prinv, pVn)
        tail_fin(pb, pq, psO, prinv, last=True)

    nc.compile()
    return nc


_NC_CACHE = None


def _get_nc():
    global _NC_CACHE
    if _NC_CACHE is None:
        _NC_CACHE = build_nc()
    return _NC_CACHE


def kernel(query: np.ndarray, value: np.ndarray) -> np.ndarray:
    query = np.asarray(query)
    value = np.asarray(value)
    assert query.shape == (B, LQ, D) and value.shape == (B, LV, D)
    q16 = np.ascontiguousarray(query.astype(np.float16))
    v16 = np.ascontiguousarray(value.astype(np.float16))
    nc = _get_nc()
    in_maps = [
        {
            "q16": q16[i * BPC : (i + 1) * BPC],
            "v16": v16[i * BPC : (i + 1) * BPC],
        }
        for i in range(NCORES)
    ]
    res = run_bass_kernel_spmd(nc, in_maps, list(range(NCORES)))
    out = np.concatenate(
        [res.results[i]["o"].astype(np.float32) for i in range(NCORES)], axis=0
    )
    return out
